# revision 1
# baseline (speedup 1.0000x reference)
"""Trainium2 Bass kernel for BaichuanAttention (hidden=5120, 40 heads, b=2, s=2048).

Tensor-parallel over heads across 8 NeuronCores: each core computes QKV for its
5 heads (sharded W_pack rows), flash-style causal attention, and a partial
o_proj (sharded W_o columns); partials are combined with an on-device
ReduceScatter and reassembled on the host.

Matmuls run as float32r (TF32-like fast fp32 path, ~1.5e-4 rel err).
"""

import math
import sys

for _p in ("/opt/trn_rl_repo",):
    if _p not in sys.path:
        sys.path.insert(0, _p)

import numpy as np

import concourse.bass as bass
import concourse.mybir as mybir
import concourse.tile as tile
from concourse import bacc, bass_utils

F32 = mybir.dt.float32
F32R = mybir.dt.float32r


class Cfg:
    def __init__(self, hidden=5120, n_heads=40, dh=128, B=2, S=2048, n_cores=8):
        self.hidden = hidden
        self.n_heads = n_heads
        self.dh = dh
        self.B = B
        self.S = S
        self.n_cores = n_cores
        assert dh == 128
        self.HL = n_heads // n_cores          # heads per core
        self.F = 3 * self.HL * dh             # per-core packed qkv rows
        self.FO = self.HL * dh                # per-core o_proj input width
        self.T = B * S                        # total tokens
        self.KT = hidden // 128               # contraction tiles for qkv
        self.TC = self.T // 512               # token chunks for qkv
        self.SQT = S // 128                   # q tiles per batch
        self.QC = S // 512                    # q chunks per batch
        self.OC = hidden // 512               # o chunks
        self.FTO = self.FO // 128             # attn feature tiles per core
        self.TG = max(1, self.T // 1024)      # reduce-scatter groups
        assert self.T % (self.TG * n_cores) == 0
        self.RS_ROWS = self.T // (self.TG * n_cores)  # out rows per core per group

    def key(self):
        return (self.hidden, self.n_heads, self.dh, self.B, self.S, self.n_cores)


def _ft_splits(n, cap=8):
    out = []
    while n > 0:
        take = min(cap, n)
        # avoid a tiny trailing pass
        if 0 < n - take < 3 and take > 4:
            take = n - 2
        out.append(take)
        n -= take
    return out


def build_program(cfg: Cfg, mode: str, phases: str = "ABC"):
    """mode: 'causal' (ignore mask input, causal skip), 'dense' (zero mask),
    'masked' (general additive mask input, pre-scaled by sqrt(dh) on host)."""
    assert mode in ("causal", "dense", "masked")
    c = cfg
    nc = bacc.Bacc("TRN2", target_bir_lowering=False, debug=False,
                   num_devices=c.n_cores)
    mask_ext = None
    xt = nc.dram_tensor("xt", [c.hidden, c.T], F32R, kind="ExternalInput").ap()
    wqkvt = nc.dram_tensor("wqkvt", [c.hidden, c.F], F32R,
                           kind="ExternalInput").ap()
    wot = nc.dram_tensor("wot", [c.FO, c.hidden], F32R,
                         kind="ExternalInput").ap()
    if mode == "masked":
        mask_ext = nc.dram_tensor("mask", [c.S, c.S], F32,
                                  kind="ExternalInput").ap()
    out_ext = nc.dram_tensor("out", [c.TG, c.RS_ROWS, c.hidden], F32,
                             kind="ExternalOutput").ap()

    inv_sqrt_dh = 1.0 / math.sqrt(c.dh)
    NEG = -1.0e9

    with tile.TileContext(nc) as tc:
        with tc.tile_pool(name="dram", bufs=1, space="DRAM") as dram:
            qkvt = dram.tile([c.F, c.T], F32R)
            partials = [dram.tile([c.T // c.TG, c.hidden], F32,
                                  tag=f"part{i}", name=f"part{i}")
                        for i in range(c.TG)]

            # ---------------- Phase A: QKV projection -------------------
            # qkvt[f, t] = sum_h wqkvt[h, f] * xt[h, t]
            do_a, do_b, do_c = ("A" in phases), ("B" in phases), ("C" in phases)
            wq_r = wqkvt.rearrange("(ko p) f -> p ko f", p=128)
            xt_r = xt.rearrange("(ko p) t -> p ko t", p=128)
            qkvt_r = qkvt.rearrange("(ft p) t -> ft p t", p=128)
            splits = _ft_splits(c.F // 128) if do_a else []
            with tc.tile_pool(name="qkv_w", bufs=1) as wpool, \
                 tc.tile_pool(name="qkv_x", bufs=6) as xpool, \
                 tc.tile_pool(name="qkv_o", bufs=8) as opool, \
                 tc.tile_pool(name="qkv_ps", bufs=8, space="PSUM") as pspool:
                ft0 = 0
                for nft in splits:
                    w_sb = wpool.tile([128, c.KT, nft * 128], F32R, tag="w")
                    for kq in range(c.KT):
                        nc.sync.dma_start(
                            w_sb[:, kq],
                            wq_r[:, kq, ft0 * 128:(ft0 + nft) * 128])
                    for tci in range(c.TC):
                        pss = [pspool.tile([128, 512], F32, tag="ps",
                                           name=f"ps{i}")
                               for i in range(nft)]
                        for k in range(c.KT):
                            x_sb = xpool.tile([128, 512], F32R, tag="x")
                            nc.sync.dma_start(
                                x_sb[:], xt_r[:, k, tci * 512:(tci + 1) * 512])
                            for i in range(nft):
                                nc.tensor.matmul(
                                    pss[i][:],
                                    w_sb[:, k, i * 128:(i + 1) * 128],
                                    x_sb[:],
                                    start=(k == 0), stop=(k == c.KT - 1))
                        for i in range(nft):
                            o_sb = opool.tile([128, 512], F32R, tag="o")
                            nc.vector.tensor_copy(o_sb[:], pss[i][:])
                            nc.sync.dma_start(
                                qkvt_r[ft0 + i, :, tci * 512:(tci + 1) * 512],
                                o_sb[:])
                    ft0 += nft

            # ---------------- Phase B: attention ------------------------
            with tc.tile_pool(name="att_at", bufs=1) as atpool:
              attnT = atpool.tile([128, c.FTO, c.T], F32R)
              with tc.tile_pool(name="att_const", bufs=1) as cpool, \
                 tc.tile_pool(name="att_in", bufs=2) as inpool, \
                 tc.tile_pool(name="att_v", bufs=1) as vpool, \
                 tc.tile_pool(name="att_p", bufs=5) as ppool, \
                 tc.tile_pool(name="att_pt", bufs=c.S // 128 + 2) as ptpool, \
                 tc.tile_pool(name="att_sm", bufs=2) as smpool, \
                 tc.tile_pool(name="att_ms", bufs=(4 if mode == "masked" else 1)) as mspool, \
                 tc.tile_pool(name="ps_s", bufs=3, space="PSUM") as ps_s, \
                 tc.tile_pool(name="ps_pt", bufs=2, space="PSUM") as ps_pt, \
                 tc.tile_pool(name="ps_at", bufs=2, space="PSUM") as ps_at, \
                 tc.tile_pool(name="ps_sm", bufs=1, space="PSUM") as ps_sm:

                ident = cpool.tile([128, 128], F32R)
                ones1 = cpool.tile([1, 128], F32R)
                with tc.tile_pool(name="att_tmp", bufs=1) as tmppool:
                    ident32 = tmppool.tile([128, 128], F32)
                    nc.gpsimd.memset(ident32[:], 0.0)
                    nc.gpsimd.affine_select(
                        out=ident32[:], in_=ident32[:],
                        compare_op=mybir.AluOpType.not_equal, fill=1.0,
                        base=0, pattern=[[-1, 128]], channel_multiplier=1)
                    nc.vector.tensor_copy(ident[:], ident32[:])
                    ones32 = tmppool.tile([1, 128], F32, tag="ones32")
                    nc.vector.memset(ones32[:], 1.0)
                    nc.vector.tensor_copy(ones1[:], ones32[:])
                cmasks = []
                if mode == "causal":
                    for off in range(4):
                        m = cpool.tile([128, 512], F32, tag=f"cm{off}",
                                       name=f"cm{off}")
                        nc.gpsimd.memset(m[:], 0.0)
                        # m[p, y] = 0 where y <= off*128 + p else NEG
                        nc.gpsimd.affine_select(
                            out=m[:], in_=m[:],
                            compare_op=mybir.AluOpType.is_ge, fill=NEG,
                            base=off * 128, pattern=[[-1, 512]],
                            channel_multiplier=1)
                        cmasks.append(m)

                for b in range(c.B if do_b else 0):
                    for h in range(c.HL):
                        q_sb = inpool.tile([128, c.S], F32R, tag="q")
                        k_sb = inpool.tile([128, c.S], F32R, tag="k")
                        v_sb = inpool.tile([128, c.S], F32R, tag="v")
                        t0 = b * c.S
                        nc.sync.dma_start(
                            q_sb[:], qkvt[h * 128:(h + 1) * 128, t0:t0 + c.S])
                        nc.sync.dma_start(
                            k_sb[:], qkvt[(c.HL + h) * 128:(c.HL + h + 1) * 128,
                                          t0:t0 + c.S])
                        nc.sync.dma_start(
                            v_sb[:], qkvt[(2 * c.HL + h) * 128:
                                          (2 * c.HL + h + 1) * 128,
                                          t0:t0 + c.S])
                        # V to token-major [128, st, dh]
                        v_tok = vpool.tile([128, c.SQT, 128], F32R)
                        for st in range(c.SQT):
                            vt_ps = ps_sm.tile([128, 128], F32R, tag="sm")
                            nc.tensor.matmul(vt_ps[:],
                                             v_sb[:, st * 128:(st + 1) * 128],
                                             ident[:], is_transpose=True)
                            nc.vector.tensor_copy(v_tok[:, st, :], vt_ps[:])

                        for qc in range(c.QC):
                            if mode == "causal":
                                nkt = 4 * (qc + 1)
                            else:
                                nkt = c.SQT
                            pts = [ptpool.tile([128, 512], F32R, tag="pt",
                                               name=f"pt{i}")
                                   for i in range(nkt)]
                            rqT_ps = ps_sm.tile([1, 512], F32R, tag="sm")
                            for qtl in range(4):
                                qt = qc * 4 + qtl
                                nkc = (qt // 4 + 1) if mode == "causal" \
                                    else c.S // 512
                                dsum = smpool.tile([128, 4], F32, tag="dsum")
                                for kc in range(nkc):
                                    s_ps = ps_s.tile([128, 512], F32, tag="s")
                                    nc.tensor.matmul(
                                        s_ps[:],
                                        q_sb[:, qt * 128:(qt + 1) * 128],
                                        k_sb[:, kc * 512:(kc + 1) * 512],
                                        start=True, stop=True)
                                    if mode == "causal" and kc == nkc - 1:
                                        nc.vector.tensor_tensor(
                                            s_ps[:], s_ps[:],
                                            cmasks[qt % 4][:],
                                            mybir.AluOpType.add)
                                    elif mode == "masked":
                                        m_sb = mspool.tile([128, 512], F32,
                                                           tag="m")
                                        nc.sync.dma_start(
                                            m_sb[:],
                                            mask_ext[qt * 128:(qt + 1) * 128,
                                                     kc * 512:(kc + 1) * 512])
                                        nc.vector.tensor_tensor(
                                            s_ps[:], s_ps[:], m_sb[:],
                                            mybir.AluOpType.add)
                                    p_sb = ppool.tile([128, 512], F32R,
                                                      tag="p")
                                    nc.scalar.activation(
                                        p_sb[:], s_ps[:],
                                        mybir.ActivationFunctionType.Exp,
                                        scale=inv_sqrt_dh,
                                        accum_out=dsum[:, kc:kc + 1])
                                    # transpose the four 128-blocks into pts
                                    for j in range(4):
                                        pt_ps = ps_pt.tile([128, 128], F32R,
                                                           tag="ptp")
                                        nc.tensor.matmul(
                                            pt_ps[:],
                                            p_sb[:, j * 128:(j + 1) * 128],
                                            ident[:], is_transpose=True)
                                        nc.vector.tensor_copy(
                                            pts[kc * 4 + j][:, qtl * 128:
                                                            (qtl + 1) * 128],
                                            pt_ps[:])
                                # 1/rowsum -> transposed into rqT_ps column
                                rqs = smpool.tile([128, 1], F32, tag="rqs")
                                nc.vector.tensor_reduce(
                                    rqs[:], dsum[:, :nkc],
                                    axis=mybir.AxisListType.X,
                                    op=mybir.AluOpType.add)
                                rq = smpool.tile([128, 1], F32, tag="rq")
                                nc.vector.reciprocal(rq[:], rqs[:])
                                rqr = smpool.tile([128, 1], F32R, tag="rqr")
                                nc.vector.tensor_copy(rqr[:], rq[:])
                                nc.tensor.matmul(
                                    rqT_ps[:, qtl * 128:(qtl + 1) * 128],
                                    rqr[:], ident[:], is_transpose=True)
                            rqT_sb = smpool.tile([1, 512], F32R, tag="rqT")
                            nc.vector.tensor_copy(rqT_sb[:], rqT_ps[:])
                            rqb_ps = ps_sm.tile([128, 512], F32, tag="sm")
                            nc.tensor.matmul(rqb_ps[:], ones1[:], rqT_sb[:],
                                             start=True, stop=True)
                            rqb_sb = smpool.tile([128, 512], F32, tag="rqb")
                            nc.vector.tensor_copy(rqb_sb[:], rqb_ps[:])
                            at_ps = ps_at.tile([128, 512], F32, tag="at")
                            for kt in range(nkt):
                                nc.tensor.matmul(
                                    at_ps[:], v_tok[:, kt, :], pts[kt][:],
                                    start=(kt == 0), stop=(kt == nkt - 1))
                            nc.vector.tensor_tensor(
                                attnT[:, h, t0 + qc * 512:t0 + (qc + 1) * 512],
                                at_ps[:], rqb_sb[:], mybir.AluOpType.mult)

              # ---------------- Phase C: o_proj + reduce-scatter ------
              wot_r = wot.rearrange("(ft p) o -> p ft o", p=128)
              with tc.tile_pool(name="op_w", bufs=3) as wopool, \
                   tc.tile_pool(name="op_o", bufs=6) as oopool, \
                   tc.tile_pool(name="op_ps", bufs=4, space="PSUM") as opps:
                  tt_per_g = c.T // c.TG // 128
                  for tg in range(c.TG if do_c else 0):
                      for oc in range(c.OC):
                          wo_sb = wopool.tile([128, c.FTO, 512], F32R,
                                              tag="wo")
                          nc.sync.dma_start(
                              wo_sb[:],
                              wot_r[:, :, oc * 512:(oc + 1) * 512])
                          for tl in range(tt_per_g):
                              tt = tg * tt_per_g + tl
                              ps = opps.tile([128, 512], F32, tag="ops")
                              for ft in range(c.FTO):
                                  nc.tensor.matmul(
                                      ps[:],
                                      attnT[:, ft, tt * 128:(tt + 1) * 128],
                                      wo_sb[:, ft, :],
                                      start=(ft == 0),
                                      stop=(ft == c.FTO - 1))
                              po_sb = oopool.tile([128, 512], F32, tag="po")
                              nc.vector.tensor_copy(po_sb[:], ps[:])
                              nc.sync.dma_start(
                                  partials[tg][tl * 128:(tl + 1) * 128,
                                               oc * 512:(oc + 1) * 512],
                                  po_sb[:])
                      rs_out = dram.tile([c.RS_ROWS, c.hidden], F32,
                                         tag="rs")
                      nc.gpsimd.collective_compute(
                          "ReduceScatter",
                          mybir.AluOpType.add,
                          replica_groups=[list(range(c.n_cores))],
                          ins=[partials[tg][:].opt()],
                          outs=[rs_out[:].opt()],
                      )
                      nc.gpsimd.dma_start(out_ext[tg], rs_out[:])

    nc.compile()
    return nc


# --------------------------------------------------------------------------
_CACHE = {}


def _get_program(cfg: Cfg, mode: str):
    key = (cfg.key(), mode)
    if key not in _CACHE:
        _CACHE[key] = build_program(cfg, mode)
    return _CACHE[key]


def prepare_inputs(cfg: Cfg, hidden_states, attention_mask, W_pack, W_o):
    """Host-side shard + layout prep. Returns (mode, in_maps)."""
    c = cfg
    X = np.asarray(hidden_states, dtype=np.float32).reshape(c.T, c.hidden)
    XT = np.ascontiguousarray(X.T)

    mask = np.asarray(attention_mask, dtype=np.float32).reshape(c.S, c.S)
    causal_ref = np.where(
        np.tril(np.ones((c.S, c.S), dtype=bool)), 0.0, -1e9
    ).astype(np.float32)
    if np.array_equal(mask, causal_ref):
        mode = "causal"
    elif not mask.any():
        mode = "dense"
    else:
        mode = "masked"

    W_pack = np.asarray(W_pack, dtype=np.float32)
    W_o = np.asarray(W_o, dtype=np.float32)
    H = c.hidden
    in_maps = []
    for g in range(c.n_cores):
        r0, r1 = g * c.FO, (g + 1) * c.FO
        wq = W_pack[r0:r1]
        wk = W_pack[H + r0:H + r1]
        wv = W_pack[2 * H + r0:2 * H + r1]
        wqkvT = np.ascontiguousarray(
            np.concatenate([wq, wk, wv], axis=0).T)       # [H, F]
        woT = np.ascontiguousarray(W_o[:, r0:r1].T)       # [FO, H]
        m = {"xt": XT, "wqkvt": wqkvT, "wot": woT}
        if mode == "masked":
            m["mask"] = np.ascontiguousarray(mask * math.sqrt(c.dh))
        in_maps.append(m)
    return mode, in_maps


def assemble_output(cfg: Cfg, results):
    c = cfg
    full = np.empty((c.T, c.hidden), dtype=np.float32)
    rows_g = c.T // c.TG
    for g in range(c.n_cores):
        o = results[g]["out"].reshape(c.TG, c.RS_ROWS, c.hidden)
        for tg in range(c.TG):
            a = tg * rows_g + g * c.RS_ROWS
            full[a:a + c.RS_ROWS] = o[tg]
    return full.reshape(c.B, c.S, c.hidden)


def kernel(hidden_states, attention_mask, W_pack, W_o):
    cfg = Cfg()
    mode, in_maps = prepare_inputs(cfg, hidden_states, attention_mask,
                                   W_pack, W_o)
    nc = _get_program(cfg, mode)
    res = bass_utils.run_bass_kernel_spmd(nc, in_maps,
                                          list(range(cfg.n_cores)))
    return assemble_output(cfg, res.results)



# revision 8
# speedup vs baseline: 1.3770x; 1.3770x over previous
"""Trainium2 Bass kernel for BaichuanAttention (hidden=5120, 40 heads, b=2, s=2048).

Tensor-parallel over heads across 8 NeuronCores, bf16 compute:
  A) QKV projection with SBUF-resident bf16 weights, X streamed.
  B) Flash-style causal attention in S^T form (scores computed as K^T.Q so
     exp() writes P^T directly -- no P transposes), V transposed on-chip.
  C) Per-batch AllToAll of the small pre-o_proj activations (features ->
     token shards), then a local full-width o_proj per core.
Host reassembles the token-sharded outputs.
"""

import math
import sys

for _p in ("/opt/trn_rl_repo",):
    if _p not in sys.path:
        sys.path.insert(0, _p)

import numpy as np
import ml_dtypes

import concourse.bass as bass
import concourse.mybir as mybir
import concourse.tile as tile
from concourse import bacc, bass_utils

F32 = mybir.dt.float32
F32R = mybir.dt.float32r
BF16 = mybir.dt.bfloat16
BF = ml_dtypes.bfloat16


class Cfg:
    def __init__(self, hidden=5120, n_heads=40, dh=128, B=2, S=2048, n_cores=8):
        self.hidden = hidden
        self.n_heads = n_heads
        self.dh = dh
        self.B = B
        self.S = S
        self.n_cores = n_cores
        assert dh == 128
        self.HL = n_heads // n_cores          # heads per core (5)
        self.F = 3 * self.HL * dh             # per-core packed qkv rows (1920)
        self.FO = self.HL * dh                # per-core attn feature width (640)
        self.T = B * S                        # total tokens (4096)
        self.KC = hidden // 128               # contraction chunks (40)
        self.TC = self.T // 512               # token chunks for qkv (8)
        self.SKT = S // 128                   # k tiles per batch seq (16)
        self.QC = S // 512                    # q chunks per batch (4)
        self.NFT = self.F // 128              # qkv feature tiles (15)
        self.TSH = S // n_cores               # token shard per core per batch (256)
        self.OC = hidden // 512               # o_proj out chunks (10)

    def key(self):
        return (self.hidden, self.n_heads, self.dh, self.B, self.S, self.n_cores)


def build_program(cfg: Cfg, mode: str):
    """mode: 'causal' (mult-mask diag blocks + block skip), 'dense' (no mask),
    'masked' (general additive mask, host passes maskT pre-scaled)."""
    assert mode in ("causal", "dense", "masked")
    c = cfg
    nc = bacc.Bacc("TRN2", target_bir_lowering=False, debug=False,
                   num_devices=c.n_cores)

    xt = nc.dram_tensor("xt", [c.hidden, c.T], BF16, kind="ExternalInput").ap()
    wqkvt = nc.dram_tensor("wqkvt", [c.hidden, c.F], BF16,
                           kind="ExternalInput").ap()
    wot = nc.dram_tensor("wot", [c.hidden, c.hidden], BF16,
                         kind="ExternalInput").ap()
    mask_ext = None
    if mode == "masked":
        mask_ext = nc.dram_tensor("maskt", [c.S, c.S], F32,
                                  kind="ExternalInput").ap()
    # per-core output: for each batch, this core's token shard (all hidden)
    out_ext = nc.dram_tensor("out", [c.B, c.TSH, c.hidden], F32,
                             kind="ExternalOutput").ap()

    inv_sqrt_dh = 1.0 / math.sqrt(c.dh)

    xt_r = xt.rearrange("(kc p) t -> p kc t", p=128)
    wq_r = wqkvt.rearrange("(kc p) f -> p kc f", p=128)
    wo_r = wot.rearrange("(kc p) j -> p kc j", p=128)

    with tile.TileContext(nc) as tc:
        with tc.tile_pool(name="dram", bufs=1, space="DRAM") as dram:
            qkv = dram.tile([c.NFT, 128, c.T], BF16)

            # ---------------- Phase A: QKV projection -------------------
            # qkv[ft, d, t] = sum_h W[h, ft*128+d] * X[h, t]   (feature-major)
            splits = [8, 7] if c.NFT == 15 else [c.NFT]
            with tc.tile_pool(name="qkv_w", bufs=1) as wpool, \
                 tc.tile_pool(name="qkv_x", bufs=2) as xpool, \
                 tc.tile_pool(name="qkv_o", bufs=8) as opool, \
                 tc.tile_pool(name="qkv_ps", bufs=8, space="PSUM") as pspool:
                ft0 = 0
                for nft in splits:
                    w_sb = wpool.tile([128, c.KC, max(splits) * 128], BF16,
                                      tag="w")
                    nc.sync.dma_start(
                        w_sb[:, :, :nft * 128],
                        wq_r[:, :, ft0 * 128:(ft0 + nft) * 128])
                    for tci in range(c.TC):
                        x_sb = xpool.tile([128, c.KC, 512], BF16, tag="x")
                        nc.sync.dma_start(
                            x_sb[:], xt_r[:, :, tci * 512:(tci + 1) * 512])
                        for i in range(nft):
                            ps = pspool.tile([128, 512], F32, tag="ps")
                            for kc in range(c.KC):
                                nc.tensor.matmul(
                                    ps[:],
                                    w_sb[:, kc, i * 128:(i + 1) * 128],
                                    x_sb[:, kc, :],
                                    start=(kc == 0), stop=(kc == c.KC - 1))
                            o_sb = opool.tile([128, 512], BF16, tag="o")
                            nc.scalar.copy(o_sb[:], ps[:])
                            nc.sync.dma_start(
                                qkv[ft0 + i, :, tci * 512:(tci + 1) * 512],
                                o_sb[:])
                    ft0 += nft

            # ---------------- Phase B + C (interleaved per batch) -------
            with tc.tile_pool(name="att_const", bufs=1) as cpool, \
                 tc.tile_pool(name="att_at", bufs=2) as atpool, \
                 tc.tile_pool(name="att_in", bufs=2) as inpool, \
                 tc.tile_pool(name="att_v", bufs=2) as vpool, \
                 tc.tile_pool(name="att_pt", bufs=c.SKT + 1) as ptpool, \
                 tc.tile_pool(name="att_acc", bufs=2) as accpool, \
                 tc.tile_pool(name="att_sm", bufs=3) as smpool, \
                 tc.tile_pool(name="att_ms", bufs=(4 if mode == "masked" else 1)) as mspool, \
                 tc.tile_pool(name="op_attn", bufs=1) as apool, \
                 tc.tile_pool(name="op_w", bufs=2) as wopool, \
                 tc.tile_pool(name="op_o", bufs=2) as oopool, \
                 tc.tile_pool(name="ps_s", bufs=2, space="PSUM") as ps_s, \
                 tc.tile_pool(name="ps_at", bufs=2, space="PSUM") as ps_at, \
                 tc.tile_pool(name="ps_ms", bufs=2, space="PSUM") as ps_ms, \
                 tc.tile_pool(name="ps_op", bufs=2, space="PSUM") as ps_op:

                # constants
                ident = cpool.tile([128, 128], BF16)
                ones_col = cpool.tile([128, 1], F32R)   # lhsT for colsum
                ones_row = cpool.tile([1, 128], F32R)   # lhsT for broadcast
                with tc.tile_pool(name="att_tmp", bufs=1) as tmppool:
                    t32 = tmppool.tile([128, 128], F32)
                    nc.gpsimd.memset(t32[:], 0.0)
                    nc.gpsimd.affine_select(
                        out=t32[:], in_=t32[:],
                        compare_op=mybir.AluOpType.not_equal, fill=1.0,
                        base=0, pattern=[[-1, 128]], channel_multiplier=1)
                    nc.vector.tensor_copy(ident[:], t32[:])
                    o32 = tmppool.tile([128, 1], F32, tag="o32")
                    nc.vector.memset(o32[:], 1.0)
                    nc.vector.tensor_copy(ones_col[:], o32[:])
                    r32 = tmppool.tile([1, 128], F32, tag="r32")
                    nc.vector.memset(r32[:], 1.0)
                    nc.vector.tensor_copy(ones_row[:], r32[:])
                cmasks = []
                if mode == "causal":
                    # multiplicative masks for the 4 diagonal k-tiles of each
                    # 512-wide q chunk: m[p, y] = 1 where y >= p + off*128
                    with tc.tile_pool(name="att_cm", bufs=2) as cmtmp:
                        for off in range(4):
                            m32 = cmtmp.tile([128, 512], F32, tag="m32",
                                             name=f"m32_{off}")
                            nc.gpsimd.memset(m32[:], 1.0)
                            # keep 1.0 where (y - off*128 - p) >= 0 else 0
                            nc.gpsimd.affine_select(
                                out=m32[:], in_=m32[:],
                                compare_op=mybir.AluOpType.is_ge, fill=0.0,
                                base=-off * 128, pattern=[[1, 512]],
                                channel_multiplier=-1)
                            m = cpool.tile([128, 512], BF16, tag=f"cm{off}",
                                           name=f"cm{off}")
                            nc.vector.tensor_copy(m[:], m32[:])
                            cmasks.append(m)

                def attend_head(b, h, attnT):
                    t0 = b * c.S
                    q_sb = inpool.tile([128, c.S], BF16, tag="q")
                    k_sb = inpool.tile([128, c.S], BF16, tag="k")
                    v_sb = inpool.tile([128, c.S], BF16, tag="v")
                    nc.sync.dma_start(q_sb[:], qkv[h, :, t0:t0 + c.S])
                    nc.sync.dma_start(k_sb[:], qkv[c.HL + h, :, t0:t0 + c.S])
                    nc.sync.dma_start(v_sb[:],
                                      qkv[2 * c.HL + h, :, t0:t0 + c.S])
                    # V to token-major [tok_p, st, dh]
                    v_tok = vpool.tile([128, c.SKT, 128], BF16, tag="vt")
                    for st in range(c.SKT):
                        vt_ps = ps_ms.tile([128, 128], BF16, tag="ms")
                        nc.tensor.matmul(vt_ps[:],
                                         v_sb[:, st * 128:(st + 1) * 128],
                                         ident[:], is_transpose=True)
                        nc.vector.tensor_copy(v_tok[:, st, :], vt_ps[:])

                    for qc in range(c.QC):
                        nkt = 4 * (qc + 1) if mode == "causal" else c.SKT
                        acc = accpool.tile([128, 512], F32R, tag="acc")
                        pts = []
                        for kt in range(nkt):
                            s_ps = ps_s.tile([128, 512], F32, tag="s")
                            nc.tensor.matmul(
                                s_ps[:],
                                k_sb[:, kt * 128:(kt + 1) * 128],
                                q_sb[:, qc * 512:(qc + 1) * 512],
                                start=True, stop=True)
                            if mode == "masked":
                                m_sb = mspool.tile([128, 512], F32, tag="m")
                                nc.sync.dma_start(
                                    m_sb[:],
                                    mask_ext[kt * 128:(kt + 1) * 128,
                                             qc * 512:(qc + 1) * 512])
                                nc.vector.tensor_tensor(
                                    s_ps[:], s_ps[:], m_sb[:],
                                    mybir.AluOpType.add)
                            pt = ptpool.tile([128, 512], BF16, tag="pt")
                            nc.scalar.activation(
                                pt[:], s_ps[:],
                                mybir.ActivationFunctionType.Exp,
                                scale=inv_sqrt_dh)
                            if mode == "causal" and kt >= 4 * qc:
                                nc.vector.tensor_tensor(
                                    pt[:], pt[:], cmasks[kt - 4 * qc][:],
                                    mybir.AluOpType.mult)
                            # denominator accumulation (gpsimd, f32)
                            if kt == 0:
                                nc.gpsimd.tensor_copy(acc[:], pt[:])
                            else:
                                nc.gpsimd.tensor_tensor(
                                    acc[:], acc[:], pt[:],
                                    mybir.AluOpType.add)
                            pts.append(pt)
                        # denom row -> reciprocal -> broadcast
                        den_ps = ps_ms.tile([1, 512], F32, tag="ms")
                        nc.tensor.matmul(den_ps[:], ones_col[:], acc[:],
                                         start=True, stop=True)
                        rden = smpool.tile([1, 512], F32, tag="rden")
                        nc.vector.reciprocal(rden[:], den_ps[:])
                        rdenr = smpool.tile([1, 512], F32R, tag="rdenr")
                        nc.vector.tensor_copy(rdenr[:], rden[:])
                        bc_ps = ps_ms.tile([128, 512], F32, tag="ms")
                        nc.tensor.matmul(bc_ps[:], ones_row[:], rdenr[:],
                                         start=True, stop=True)
                        bc_sb = smpool.tile([128, 512], F32, tag="bc")
                        nc.vector.tensor_copy(bc_sb[:], bc_ps[:])
                        # PV
                        at_ps = ps_at.tile([128, 512], F32, tag="at")
                        for kt in range(nkt):
                            nc.tensor.matmul(
                                at_ps[:], v_tok[:, kt, :], pts[kt][:],
                                start=(kt == 0), stop=(kt == nkt - 1))
                        nc.vector.tensor_tensor(
                            attnT[:, h, qc * 512:(qc + 1) * 512],
                            at_ps[:], bc_sb[:], mybir.AluOpType.mult)

                def o_proj_batch(b, attnT):
                    # AllToAll: features (this core's heads, all tokens) ->
                    # token shard (all features)
                    a2a_in = dram.tile([c.n_cores, c.FO, c.TSH], BF16,
                                       tag=f"a2a_in{b}", name=f"a2a_in{b}")
                    a2a_out = dram.tile([c.n_cores, c.FO, c.TSH], BF16,
                                        tag=f"a2a_out{b}", name=f"a2a_out{b}")
                    for g in range(c.n_cores):
                        nc.sync.dma_start(
                            a2a_in[g].rearrange("(ft p) t -> p ft t", p=128),
                            attnT[:, :, g * c.TSH:(g + 1) * c.TSH])
                    nc.gpsimd.collective_compute(
                        "AllToAll",
                        mybir.AluOpType.bypass,
                        replica_groups=[list(range(c.n_cores))],
                        ins=[a2a_in[:].opt()],
                        outs=[a2a_out[:].opt()],
                    )
                    # gathered activations: [5120 feats, TSH tokens]
                    attn_sb = apool.tile([128, c.KC, c.TSH], BF16, tag="ag")
                    nc.sync.dma_start(
                        attn_sb[:],
                        a2a_out.rearrange("s (ft p) t -> p (s ft) t", p=128))
                    KH = c.KC // 2
                    for oc in range(c.OC):
                        wo_sbs = []
                        for half in range(2):
                            wo_sb = wopool.tile([128, KH, 512], BF16,
                                                tag="wo")
                            nc.sync.dma_start(
                                wo_sb[:],
                                wo_r[:, half * KH:(half + 1) * KH,
                                     oc * 512:(oc + 1) * 512])
                            wo_sbs.append(wo_sb)
                        for tt in range(c.TSH // 128):
                            ps = ps_op.tile([128, 512], F32, tag="ops")
                            for half in range(2):
                                for k in range(KH):
                                    nc.tensor.matmul(
                                        ps[:],
                                        attn_sb[:, half * KH + k,
                                                tt * 128:(tt + 1) * 128],
                                        wo_sbs[half][:, k, :],
                                        start=(half == 0 and k == 0),
                                        stop=(half == 1 and k == KH - 1))
                            po_sb = oopool.tile([128, 512], F32, tag="po")
                            nc.vector.tensor_copy(po_sb[:], ps[:])
                            nc.sync.dma_start(
                                out_ext[b, tt * 128:(tt + 1) * 128,
                                        oc * 512:(oc + 1) * 512],
                                po_sb[:])

                for b in range(c.B):
                    attnT = atpool.tile([128, c.HL, c.S], BF16, tag="attnT",
                                        name=f"attnT{b}")
                    for h in range(c.HL):
                        attend_head(b, h, attnT)
                    o_proj_batch(b, attnT)

    nc.compile()
    return nc


# --------------------------------------------------------------------------
_CACHE = {}


def _get_program(cfg: Cfg, mode: str):
    key = (cfg.key(), mode)
    if key not in _CACHE:
        _CACHE[key] = build_program(cfg, mode)
    return _CACHE[key]


def prepare_inputs(cfg: Cfg, hidden_states, attention_mask, W_pack, W_o):
    """Host-side shard + layout prep (bf16 cast). Returns (mode, in_maps)."""
    c = cfg
    X = np.asarray(hidden_states, dtype=np.float32).reshape(c.T, c.hidden)
    XT = np.ascontiguousarray(X.T).astype(BF)

    mask = np.asarray(attention_mask, dtype=np.float32).reshape(c.S, c.S)
    causal_ref = np.where(
        np.tril(np.ones((c.S, c.S), dtype=bool)), 0.0, -1e9
    ).astype(np.float32)
    if np.array_equal(mask, causal_ref):
        mode = "causal"
    elif not mask.any():
        mode = "dense"
    else:
        mode = "masked"

    W_pack = np.asarray(W_pack, dtype=np.float32)
    W_o = np.asarray(W_o, dtype=np.float32)
    H = c.hidden
    woT = np.ascontiguousarray(W_o.T).astype(BF)   # [feat, out] full
    in_maps = []
    for g in range(c.n_cores):
        r0, r1 = g * c.FO, (g + 1) * c.FO
        wq = W_pack[r0:r1]
        wk = W_pack[H + r0:H + r1]
        wv = W_pack[2 * H + r0:2 * H + r1]
        wqkvT = np.ascontiguousarray(
            np.concatenate([wq, wk, wv], axis=0).T).astype(BF)  # [H, F]
        m = {"xt": XT, "wqkvt": wqkvT, "wot": woT}
        if mode == "masked":
            m["maskt"] = np.ascontiguousarray(mask.T * math.sqrt(c.dh))
        in_maps.append(m)
    return mode, in_maps


def assemble_output(cfg: Cfg, results):
    c = cfg
    full = np.empty((c.B, c.S, c.hidden), dtype=np.float32)
    for g in range(c.n_cores):
        o = results[g]["out"].reshape(c.B, c.TSH, c.hidden)
        for b in range(c.B):
            full[b, g * c.TSH:(g + 1) * c.TSH] = o[b]
    return full


def kernel(hidden_states, attention_mask, W_pack, W_o):
    cfg = Cfg()
    mode, in_maps = prepare_inputs(cfg, hidden_states, attention_mask,
                                   W_pack, W_o)
    nc = _get_program(cfg, mode)
    res = bass_utils.run_bass_kernel_spmd(nc, in_maps,
                                          list(range(cfg.n_cores)))
    return assemble_output(cfg, res.results)


# revision 14
# speedup vs baseline: 1.4229x; 1.0333x over previous
"""Trainium2 Bass kernel for BaichuanAttention (hidden=5120, 40 heads, b=2, s=2048).

Tensor-parallel over heads across 8 NeuronCores, bf16 compute:
  A) QKV projection with SBUF-resident bf16 weights, X streamed.
  B) Flash-style causal attention in S^T form (scores computed as K^T.Q so
     exp() writes P^T directly -- no P transposes), V transposed on-chip.
  C) Per-batch AllToAll of the small pre-o_proj activations (features ->
     token shards), then a local full-width o_proj per core.
Host reassembles the token-sharded outputs.
"""

import math
import sys

for _p in ("/opt/trn_rl_repo",):
    if _p not in sys.path:
        sys.path.insert(0, _p)

import numpy as np
import ml_dtypes

import concourse.bass as bass
import concourse.mybir as mybir
import concourse.tile as tile
from concourse import bacc, bass_utils

F32 = mybir.dt.float32
F32R = mybir.dt.float32r
BF16 = mybir.dt.bfloat16
BF = ml_dtypes.bfloat16


class Cfg:
    def __init__(self, hidden=5120, n_heads=40, dh=128, B=2, S=2048, n_cores=8):
        self.hidden = hidden
        self.n_heads = n_heads
        self.dh = dh
        self.B = B
        self.S = S
        self.n_cores = n_cores
        assert dh == 128
        self.HL = n_heads // n_cores          # heads per core (5)
        self.F = 3 * self.HL * dh             # per-core packed qkv rows (1920)
        self.FO = self.HL * dh                # per-core attn feature width (640)
        self.T = B * S                        # total tokens (4096)
        self.KC = hidden // 128               # contraction chunks (40)
        self.TC = self.T // 512               # token chunks for qkv (8)
        self.SKT = S // 128                   # k tiles per batch seq (16)
        self.QC = S // 512                    # q chunks per batch (4)
        self.NFT = self.F // 128              # qkv feature tiles (15)
        self.TSH = S // n_cores               # token shard per core per batch (256)
        self.OC = hidden // 512               # o_proj out chunks (10)

    def key(self):
        return (self.hidden, self.n_heads, self.dh, self.B, self.S, self.n_cores)


def build_program(cfg: Cfg, mode: str, dbg: bool = False):
    """mode: 'causal' (mult-mask diag blocks + block skip), 'dense' (no mask),
    'masked' (general additive mask, host passes maskT pre-scaled)."""
    assert mode in ("causal", "dense", "masked")
    c = cfg
    nc = bacc.Bacc("TRN2", target_bir_lowering=False, debug=False,
                   num_devices=c.n_cores)
    dbg_ext = None
    if dbg:
        dbg_ext = nc.dram_tensor("dbg", [128, c.HL, c.S], F32,
                                 kind="ExternalOutput").ap()

    xt = nc.dram_tensor("xt", [c.hidden, c.T], BF16, kind="ExternalInput").ap()
    wqkvt = nc.dram_tensor("wqkvt", [c.hidden, c.F], BF16,
                           kind="ExternalInput").ap()
    wot = nc.dram_tensor("wot", [c.hidden, c.hidden], BF16,
                         kind="ExternalInput").ap()
    mask_ext = None
    if mode == "masked":
        mask_ext = nc.dram_tensor("maskt", [c.S, c.S], F32,
                                  kind="ExternalInput").ap()
    # per-core output: for each batch, this core's token shard (all hidden)
    out_ext = nc.dram_tensor("out", [c.B, c.TSH, c.hidden], F32,
                             kind="ExternalOutput").ap()

    inv_sqrt_dh = 1.0 / math.sqrt(c.dh)

    xt_r = xt.rearrange("(kc p) t -> p kc t", p=128)
    wq_r = wqkvt.rearrange("(kc p) f -> p kc f", p=128)
    wo_r = wot.rearrange("(kc p) j -> p kc j", p=128)

    with tile.TileContext(nc) as tc:
        with tc.tile_pool(name="dram", bufs=1, space="DRAM") as dram:
            qkv = dram.tile([c.NFT, 128, c.T], BF16)

            # ---------------- Phase A: QKV projection -------------------
            # qkv[ft, d, t] = sum_h W[h, ft*128+d] * X[h, t]   (feature-major)
            splits = [8, 7] if c.NFT == 15 else [c.NFT]
            with tc.tile_pool(name="qkv_w", bufs=1) as wpool, \
                 tc.tile_pool(name="qkv_x", bufs=2) as xpool, \
                 tc.tile_pool(name="qkv_o", bufs=8) as opool, \
                 tc.tile_pool(name="qkv_ps", bufs=8, space="PSUM") as pspool:
                assert c.KC % 4 == 0
                KQ = c.KC // 4
                ft0 = 0
                for nft in splits:
                    wts = None
                    for tci in range(c.TC):
                        xq = [xpool.tile([128, KQ, 512], BF16, tag=f"x{j}",
                                         name=f"x{j}") for j in range(4)]
                        for j in range(4):
                            nc.sync.dma_start(
                                xq[j][:],
                                xt_r[:, j * KQ:(j + 1) * KQ,
                                     tci * 512:(tci + 1) * 512])
                        if tci == 0:
                            # per-kc weight tiles: lets the next group's
                            # weight loads overlap this group's tail
                            wts = []
                            for kc in range(c.KC):
                                w_t = wpool.tile([128, max(splits) * 128],
                                                 BF16, tag=f"w{kc}",
                                                 name=f"w{kc}")
                                nc.sync.dma_start(
                                    w_t[:, :nft * 128],
                                    wq_r[:, kc,
                                         ft0 * 128:(ft0 + nft) * 128])
                                wts.append(w_t)
                        pss = [pspool.tile([128, 512], F32, tag="ps",
                                           name=f"ps{i}")
                               for i in range(nft)]
                        for kc in range(c.KC):
                            for i in range(nft):
                                nc.tensor.matmul(
                                    pss[i][:],
                                    wts[kc][:, i * 128:(i + 1) * 128],
                                    xq[kc // KQ][:, kc % KQ, :],
                                    start=(kc == 0), stop=(kc == c.KC - 1))
                        for i in range(nft):
                            o_sb = opool.tile([128, 512], BF16, tag="o")
                            nc.vector.tensor_copy(o_sb[:], pss[i][:])
                            nc.sync.dma_start(
                                qkv[ft0 + i, :, tci * 512:(tci + 1) * 512],
                                o_sb[:])
                    ft0 += nft

            # ---------------- Phase B + C (interleaved per batch) -------
            with tc.tile_pool(name="att_const", bufs=1) as cpool, \
                 tc.tile_pool(name="att_at", bufs=2) as atpool, \
                 tc.tile_pool(name="att_in", bufs=2) as inpool, \
                 tc.tile_pool(name="att_v", bufs=2) as vpool, \
                 tc.tile_pool(name="att_pt", bufs=c.SKT + 1) as ptpool, \
                 tc.tile_pool(name="att_acc", bufs=2) as accpool, \
                 tc.tile_pool(name="att_sm", bufs=3) as smpool, \
                 tc.tile_pool(name="att_ms", bufs=(4 if mode == "masked" else 1)) as mspool, \
                 tc.tile_pool(name="op_attn", bufs=1) as apool, \
                 tc.tile_pool(name="op_w", bufs=2) as wopool, \
                 tc.tile_pool(name="op_o", bufs=2) as oopool, \
                 tc.tile_pool(name="ps_s", bufs=2, space="PSUM") as ps_s, \
                 tc.tile_pool(name="ps_at", bufs=2, space="PSUM") as ps_at, \
                 tc.tile_pool(name="ps_ms", bufs=2, space="PSUM") as ps_ms, \
                 tc.tile_pool(name="ps_op", bufs=2, space="PSUM") as ps_op:

                # constants
                ident = cpool.tile([128, 128], BF16)
                ones_col = cpool.tile([128, 1], F32R)   # lhsT for colsum
                ones_row = cpool.tile([1, 128], F32R)   # lhsT for broadcast
                with tc.tile_pool(name="att_tmp", bufs=1) as tmppool:
                    t32 = tmppool.tile([128, 128], F32)
                    nc.gpsimd.memset(t32[:], 0.0)
                    nc.gpsimd.affine_select(
                        out=t32[:], in_=t32[:],
                        compare_op=mybir.AluOpType.not_equal, fill=1.0,
                        base=0, pattern=[[-1, 128]], channel_multiplier=1)
                    nc.vector.tensor_copy(ident[:], t32[:])
                    o32 = tmppool.tile([128, 1], F32, tag="o32")
                    nc.vector.memset(o32[:], 1.0)
                    nc.vector.tensor_copy(ones_col[:], o32[:])
                    r32 = tmppool.tile([1, 128], F32, tag="r32")
                    nc.vector.memset(r32[:], 1.0)
                    nc.vector.tensor_copy(ones_row[:], r32[:])
                ctri = None
                if mode == "causal":
                    # multiplicative triangle mask [128k, 128q]:
                    # m[p, y] = 1 where y >= p else 0
                    with tc.tile_pool(name="att_cm", bufs=1) as cmtmp:
                        m32 = cmtmp.tile([128, 128], F32, tag="m32")
                        nc.gpsimd.memset(m32[:], 1.0)
                        nc.gpsimd.affine_select(
                            out=m32[:], in_=m32[:],
                            compare_op=mybir.AluOpType.is_ge, fill=0.0,
                            base=0, pattern=[[1, 128]],
                            channel_multiplier=-1)
                        ctri = cpool.tile([128, 128], BF16, tag="ctri")
                        nc.vector.tensor_copy(ctri[:], m32[:])

                def attend_head(b, h, attnT):
                    t0 = b * c.S
                    q_sb = inpool.tile([128, c.S], BF16, tag="q")
                    k_sb = inpool.tile([128, c.S], BF16, tag="k")
                    v_sb = inpool.tile([128, c.S], BF16, tag="v")
                    nc.sync.dma_start(q_sb[:], qkv[h, :, t0:t0 + c.S])
                    nc.sync.dma_start(k_sb[:], qkv[c.HL + h, :, t0:t0 + c.S])
                    nc.sync.dma_start(v_sb[:],
                                      qkv[2 * c.HL + h, :, t0:t0 + c.S])
                    # V to token-major [tok_p, st, dh]
                    v_tok = vpool.tile([128, c.SKT, 128], BF16, tag="vt")
                    for st in range(c.SKT):
                        vt_ps = ps_ms.tile([128, 128], BF16, tag="ms")
                        nc.tensor.matmul(vt_ps[:],
                                         v_sb[:, st * 128:(st + 1) * 128],
                                         ident[:], is_transpose=True)
                        nc.vector.tensor_copy(v_tok[:, st, :], vt_ps[:])

                    for qc in range(c.QC):
                        nkt = 4 * (qc + 1) if mode == "causal" else c.SKT
                        acc_g = accpool.tile([128, 512], F32R, tag="accg")
                        acc_v = accpool.tile([128, 512], F32R, tag="accv")
                        pts = []
                        for kt in range(nkt):
                            off = kt - 4 * qc  # >=0: diagonal tile (causal)
                            pt = ptpool.tile([128, 512], BF16, tag="pt")
                            s_ps = ps_s.tile([128, 512], F32, tag="s")
                            if mode == "causal" and off > 0:
                                # valid q range is [off*128, 512)
                                w = 512 - off * 128
                                nc.tensor.matmul(
                                    s_ps[:, :w],
                                    k_sb[:, kt * 128:(kt + 1) * 128],
                                    q_sb[:, qc * 512 + off * 128:
                                         (qc + 1) * 512],
                                    start=True, stop=True)
                                nc.vector.memset(pt[:, :off * 128], 0.0)
                                nc.scalar.activation(
                                    pt[:, off * 128:], s_ps[:, :w],
                                    mybir.ActivationFunctionType.Exp,
                                    scale=inv_sqrt_dh)
                                nc.vector.tensor_tensor(
                                    pt[:, off * 128:(off + 1) * 128],
                                    pt[:, off * 128:(off + 1) * 128],
                                    ctri[:], mybir.AluOpType.mult)
                            else:
                                nc.tensor.matmul(
                                    s_ps[:],
                                    k_sb[:, kt * 128:(kt + 1) * 128],
                                    q_sb[:, qc * 512:(qc + 1) * 512],
                                    start=True, stop=True)
                                if mode == "masked":
                                    m_sb = mspool.tile([128, 512], F32,
                                                       tag="m")
                                    nc.sync.dma_start(
                                        m_sb[:],
                                        mask_ext[kt * 128:(kt + 1) * 128,
                                                 qc * 512:(qc + 1) * 512])
                                    nc.vector.tensor_tensor(
                                        s_ps[:], s_ps[:], m_sb[:],
                                        mybir.AluOpType.add)
                                nc.scalar.activation(
                                    pt[:], s_ps[:],
                                    mybir.ActivationFunctionType.Exp,
                                    scale=inv_sqrt_dh)
                                if mode == "causal" and off == 0:
                                    nc.vector.tensor_tensor(
                                        pt[:, :128], pt[:, :128],
                                        ctri[:], mybir.AluOpType.mult)
                            # denominator: two parallel accumulation chains
                            if kt % 2 == 0:
                                if kt == 0:
                                    nc.gpsimd.tensor_copy(acc_g[:], pt[:])
                                else:
                                    nc.gpsimd.tensor_tensor(
                                        acc_g[:], acc_g[:], pt[:],
                                        mybir.AluOpType.add)
                            else:
                                if kt == 1:
                                    nc.vector.tensor_copy(acc_v[:], pt[:])
                                else:
                                    nc.vector.tensor_tensor(
                                        acc_v[:], acc_v[:], pt[:],
                                        mybir.AluOpType.add)
                            pts.append(pt)
                        # PV first: keeps the in-order tensor queue busy
                        at_ps = ps_at.tile([128, 512], F32, tag="at")
                        for kt in range(nkt):
                            nc.tensor.matmul(
                                at_ps[:], v_tok[:, kt, :], pts[kt][:],
                                start=(kt == 0), stop=(kt == nkt - 1))
                        # denom row -> broadcast -> reciprocal (wide)
                        nc.vector.tensor_tensor(acc_v[:], acc_v[:], acc_g[:],
                                                mybir.AluOpType.add)
                        den_ps = ps_ms.tile([1, 512], F32, tag="ms")
                        nc.tensor.matmul(den_ps[:], ones_col[:], acc_v[:],
                                         start=True, stop=True)
                        den_r = smpool.tile([1, 512], F32R, tag="denr")
                        nc.vector.tensor_copy(den_r[:], den_ps[:])
                        bc_ps = ps_ms.tile([128, 512], F32, tag="ms")
                        nc.tensor.matmul(bc_ps[:], ones_row[:], den_r[:],
                                         start=True, stop=True)
                        rbc = smpool.tile([128, 512], F32, tag="rbc")
                        nc.vector.reciprocal(rbc[:], bc_ps[:])
                        nc.vector.tensor_tensor(
                            attnT[:, h, qc * 512:(qc + 1) * 512],
                            at_ps[:], rbc[:], mybir.AluOpType.mult)

                def o_proj_batch(b, attnT):
                    # AllToAll: features (this core's heads, all tokens) ->
                    # token shard (all features)
                    a2a_in = dram.tile([c.n_cores, c.FO, c.TSH], BF16,
                                       tag=f"a2a_in{b}", name=f"a2a_in{b}")
                    a2a_out = dram.tile([c.n_cores, c.FO, c.TSH], BF16,
                                        tag=f"a2a_out{b}", name=f"a2a_out{b}")
                    for g in range(c.n_cores):
                        nc.sync.dma_start(
                            a2a_in[g].rearrange("(ft p) t -> p ft t", p=128),
                            attnT[:, :, g * c.TSH:(g + 1) * c.TSH])
                    nc.gpsimd.collective_compute(
                        "AllToAll",
                        mybir.AluOpType.bypass,
                        replica_groups=[list(range(c.n_cores))],
                        ins=[a2a_in[:].opt()],
                        outs=[a2a_out[:].opt()],
                    )
                    # gathered activations: [5120 feats, TSH tokens]
                    attn_sb = apool.tile([128, c.KC, c.TSH], BF16, tag="ag")
                    nc.sync.dma_start(
                        attn_sb[:],
                        a2a_out.rearrange("s (ft p) t -> p (s ft) t", p=128))
                    KH = c.KC // 2
                    for oc in range(c.OC):
                        wo_sbs = []
                        for half in range(2):
                            wo_sb = wopool.tile([128, KH, 512], BF16,
                                                tag="wo")
                            nc.sync.dma_start(
                                wo_sb[:],
                                wo_r[:, half * KH:(half + 1) * KH,
                                     oc * 512:(oc + 1) * 512])
                            wo_sbs.append(wo_sb)
                        for tt in range(c.TSH // 128):
                            ps = ps_op.tile([128, 512], F32, tag="ops")
                            for half in range(2):
                                for k in range(KH):
                                    nc.tensor.matmul(
                                        ps[:],
                                        attn_sb[:, half * KH + k,
                                                tt * 128:(tt + 1) * 128],
                                        wo_sbs[half][:, k, :],
                                        start=(half == 0 and k == 0),
                                        stop=(half == 1 and k == KH - 1))
                            po_sb = oopool.tile([128, 512], F32, tag="po")
                            nc.vector.tensor_copy(po_sb[:], ps[:])
                            nc.sync.dma_start(
                                out_ext[b, tt * 128:(tt + 1) * 128,
                                        oc * 512:(oc + 1) * 512],
                                po_sb[:])

                for b in range(c.B):
                    attnT = atpool.tile([128, c.HL, c.S], BF16, tag="attnT",
                                        name=f"attnT{b}")
                    for h in range(c.HL):
                        attend_head(b, h, attnT)
                    if dbg and b == 0:
                        dbg_sb = smpool.tile([128, c.S], F32, tag="dbg")
                        for hh in range(c.HL):
                            nc.vector.tensor_copy(dbg_sb[:], attnT[:, hh, :])
                            nc.sync.dma_start(dbg_ext[:, hh, :], dbg_sb[:])
                    o_proj_batch(b, attnT)

    nc.compile()
    return nc


# --------------------------------------------------------------------------
_CACHE = {}


def _get_program(cfg: Cfg, mode: str):
    key = (cfg.key(), mode)
    if key not in _CACHE:
        _CACHE[key] = build_program(cfg, mode)
    return _CACHE[key]


def prepare_inputs(cfg: Cfg, hidden_states, attention_mask, W_pack, W_o):
    """Host-side shard + layout prep (bf16 cast). Returns (mode, in_maps)."""
    c = cfg
    X = np.asarray(hidden_states, dtype=np.float32).reshape(c.T, c.hidden)
    XT = np.ascontiguousarray(X.T).astype(BF)

    mask = np.asarray(attention_mask, dtype=np.float32).reshape(c.S, c.S)
    causal_ref = np.where(
        np.tril(np.ones((c.S, c.S), dtype=bool)), 0.0, -1e9
    ).astype(np.float32)
    if np.array_equal(mask, causal_ref):
        mode = "causal"
    elif not mask.any():
        mode = "dense"
    else:
        mode = "masked"

    W_pack = np.asarray(W_pack, dtype=np.float32)
    W_o = np.asarray(W_o, dtype=np.float32)
    H = c.hidden
    woT = np.ascontiguousarray(W_o.T).astype(BF)   # [feat, out] full
    in_maps = []
    for g in range(c.n_cores):
        r0, r1 = g * c.FO, (g + 1) * c.FO
        wq = W_pack[r0:r1]
        wk = W_pack[H + r0:H + r1]
        wv = W_pack[2 * H + r0:2 * H + r1]
        wqkvT = np.ascontiguousarray(
            np.concatenate([wq, wk, wv], axis=0).T).astype(BF)  # [H, F]
        m = {"xt": XT, "wqkvt": wqkvT, "wot": woT}
        if mode == "masked":
            m["maskt"] = np.ascontiguousarray(mask.T * math.sqrt(c.dh))
        in_maps.append(m)
    return mode, in_maps


def assemble_output(cfg: Cfg, results):
    c = cfg
    full = np.empty((c.B, c.S, c.hidden), dtype=np.float32)
    for g in range(c.n_cores):
        o = results[g]["out"].reshape(c.B, c.TSH, c.hidden)
        for b in range(c.B):
            full[b, g * c.TSH:(g + 1) * c.TSH] = o[b]
    return full


def kernel(hidden_states, attention_mask, W_pack, W_o):
    cfg = Cfg()
    mode, in_maps = prepare_inputs(cfg, hidden_states, attention_mask,
                                   W_pack, W_o)
    nc = _get_program(cfg, mode)
    res = bass_utils.run_bass_kernel_spmd(nc, in_maps,
                                          list(range(cfg.n_cores)))
    return assemble_output(cfg, res.results)


# revision 19
# speedup vs baseline: 1.5003x; 1.0544x over previous
"""Trainium2 Bass kernel for BaichuanAttention (hidden=5120, 40 heads, b=2, s=2048).

Tensor-parallel over heads across 8 NeuronCores, bf16 compute:
  A) QKV projection with SBUF-resident bf16 weights, X streamed.
  B) Flash-style causal attention in S^T form (scores computed as K^T.Q so
     exp() writes P^T directly -- no P transposes), V transposed on-chip.
  C) Per-batch AllToAll of the small pre-o_proj activations (features ->
     token shards), then a local full-width o_proj per core.
Host reassembles the token-sharded outputs.
"""

import math
import sys

for _p in ("/opt/trn_rl_repo",):
    if _p not in sys.path:
        sys.path.insert(0, _p)

import numpy as np
import ml_dtypes

import concourse.bass as bass
import concourse.mybir as mybir
import concourse.tile as tile
from concourse import bacc, bass_utils

F32 = mybir.dt.float32
F32R = mybir.dt.float32r
BF16 = mybir.dt.bfloat16
BF = ml_dtypes.bfloat16


class Cfg:
    def __init__(self, hidden=5120, n_heads=40, dh=128, B=2, S=2048, n_cores=8):
        self.hidden = hidden
        self.n_heads = n_heads
        self.dh = dh
        self.B = B
        self.S = S
        self.n_cores = n_cores
        assert dh == 128
        self.HL = n_heads // n_cores          # heads per core (5)
        self.F = 3 * self.HL * dh             # per-core packed qkv rows (1920)
        self.FO = self.HL * dh                # per-core attn feature width (640)
        self.T = B * S                        # total tokens (4096)
        self.KC = hidden // 128               # contraction chunks (40)
        self.TC = self.T // 512               # token chunks for qkv (8)
        self.SKT = S // 128                   # k tiles per batch seq (16)
        self.QC = S // 512                    # q chunks per batch (4)
        self.NFT = self.F // 128              # qkv feature tiles (15)
        self.TSH = S // n_cores               # token shard per core per batch (256)
        self.OC = hidden // 512               # o_proj out chunks (10)

    def key(self):
        return (self.hidden, self.n_heads, self.dh, self.B, self.S, self.n_cores)


def build_program(cfg: Cfg, mode: str, dbg: bool = False):
    """mode: 'causal' (mult-mask diag blocks + block skip), 'dense' (no mask),
    'masked' (general additive mask, host passes maskT pre-scaled)."""
    assert mode in ("causal", "dense", "masked")
    c = cfg
    nc = bacc.Bacc("TRN2", target_bir_lowering=False, debug=False,
                   num_devices=c.n_cores)
    dbg_ext = None
    if dbg:
        dbg_ext = nc.dram_tensor("dbg", [128, c.HL, c.S], F32,
                                 kind="ExternalOutput").ap()

    xt = nc.dram_tensor("xt", [c.hidden, c.T], BF16, kind="ExternalInput").ap()
    wqkvt = nc.dram_tensor("wqkvt", [c.hidden, c.F], BF16,
                           kind="ExternalInput").ap()
    wot = nc.dram_tensor("wot", [c.hidden, c.hidden], BF16,
                         kind="ExternalInput").ap()
    mask_ext = None
    if mode == "masked":
        mask_ext = nc.dram_tensor("maskt", [c.S, c.S], F32,
                                  kind="ExternalInput").ap()
    # per-core output: for each batch, this core's token shard (all hidden)
    out_ext = nc.dram_tensor("out", [c.B, c.TSH, c.hidden], F32,
                             kind="ExternalOutput").ap()

    inv_sqrt_dh = 1.0 / math.sqrt(c.dh)

    xt_r = xt.rearrange("(kc p) t -> p kc t", p=128)
    wq_r = wqkvt.rearrange("(kc p) f -> p kc f", p=128)
    wo_r = wot.rearrange("(kc p) j -> p kc j", p=128)

    with tile.TileContext(nc) as tc:
        with tc.tile_pool(name="dram", bufs=1, space="DRAM") as dram:
            qkv = dram.tile([c.NFT, 128, c.T], BF16)

            # ---------------- Phase A: QKV projection -------------------
            # qkv[ft, d, t] = sum_h W[h, ft*128+d] * X[h, t]   (feature-major)
            splits = [8, 7] if c.NFT == 15 else [c.NFT]
            with tc.tile_pool(name="qkv_w", bufs=1) as wpool, \
                 tc.tile_pool(name="qkv_x", bufs=2) as xpool, \
                 tc.tile_pool(name="qkv_o", bufs=8) as opool, \
                 tc.tile_pool(name="qkv_ps", bufs=8, space="PSUM") as pspool:
                assert c.KC % 4 == 0
                KQ = c.KC // 4
                ft0 = 0
                for nft in splits:
                    wts = None
                    for tci in range(c.TC):
                        xq = [xpool.tile([128, KQ, 512], BF16, tag=f"x{j}",
                                         name=f"x{j}") for j in range(4)]
                        for j in range(4):
                            nc.sync.dma_start(
                                xq[j][:],
                                xt_r[:, j * KQ:(j + 1) * KQ,
                                     tci * 512:(tci + 1) * 512])
                        if tci == 0:
                            # per-kc weight tiles: lets the next group's
                            # weight loads overlap this group's tail
                            wts = []
                            for kc in range(c.KC):
                                w_t = wpool.tile([128, max(splits) * 128],
                                                 BF16, tag=f"w{kc}",
                                                 name=f"w{kc}")
                                nc.sync.dma_start(
                                    w_t[:, :nft * 128],
                                    wq_r[:, kc,
                                         ft0 * 128:(ft0 + nft) * 128])
                                wts.append(w_t)
                        pss = [pspool.tile([128, 512], F32, tag="ps",
                                           name=f"ps{i}")
                               for i in range(nft)]
                        for kc in range(c.KC):
                            for i in range(nft):
                                nc.tensor.matmul(
                                    pss[i][:],
                                    wts[kc][:, i * 128:(i + 1) * 128],
                                    xq[kc // KQ][:, kc % KQ, :],
                                    start=(kc == 0), stop=(kc == c.KC - 1))
                        for i in range(nft):
                            o_sb = opool.tile([128, 512], BF16, tag="o")
                            nc.vector.tensor_copy(o_sb[:], pss[i][:])
                            nc.sync.dma_start(
                                qkv[ft0 + i, :, tci * 512:(tci + 1) * 512],
                                o_sb[:])
                    ft0 += nft

            # ---------------- Phase B + C (interleaved per batch) -------
            with tc.tile_pool(name="att_const", bufs=1) as cpool, \
                 tc.tile_pool(name="att_at", bufs=2) as atpool, \
                 tc.tile_pool(name="att_in", bufs=2) as inpool, \
                 tc.tile_pool(name="att_v", bufs=2) as vpool, \
                 tc.tile_pool(name="att_pt", bufs=c.SKT + 1) as ptpool, \
                 tc.tile_pool(name="att_acc", bufs=2) as accpool, \
                 tc.tile_pool(name="att_sm", bufs=3) as smpool, \
                 tc.tile_pool(name="att_ms", bufs=(4 if mode == "masked" else 1)) as mspool, \
                 tc.tile_pool(name="op_attn", bufs=1) as apool, \
                 tc.tile_pool(name="op_w", bufs=2) as wopool, \
                 tc.tile_pool(name="op_o", bufs=2) as oopool, \
                 tc.tile_pool(name="ps_s", bufs=2, space="PSUM") as ps_s, \
                 tc.tile_pool(name="ps_at", bufs=2, space="PSUM") as ps_at, \
                 tc.tile_pool(name="ps_ms", bufs=2, space="PSUM") as ps_ms, \
                 tc.tile_pool(name="ps_op", bufs=2, space="PSUM") as ps_op:

                # constants
                ident = cpool.tile([128, 128], BF16)
                ones_col = cpool.tile([128, 1], F32R)   # lhsT for colsum
                ones_row = cpool.tile([1, 128], F32R)   # lhsT for broadcast
                with tc.tile_pool(name="att_tmp", bufs=1) as tmppool:
                    t32 = tmppool.tile([128, 128], F32)
                    nc.gpsimd.memset(t32[:], 0.0)
                    nc.gpsimd.affine_select(
                        out=t32[:], in_=t32[:],
                        compare_op=mybir.AluOpType.not_equal, fill=1.0,
                        base=0, pattern=[[-1, 128]], channel_multiplier=1)
                    nc.vector.tensor_copy(ident[:], t32[:])
                    o32 = tmppool.tile([128, 1], F32, tag="o32")
                    nc.vector.memset(o32[:], 1.0)
                    nc.vector.tensor_copy(ones_col[:], o32[:])
                    r32 = tmppool.tile([1, 128], F32, tag="r32")
                    nc.vector.memset(r32[:], 1.0)
                    nc.vector.tensor_copy(ones_row[:], r32[:])
                ctri = None
                if mode == "causal":
                    # multiplicative triangle mask [128k, 128q]:
                    # m[p, y] = 1 where y >= p else 0
                    with tc.tile_pool(name="att_cm", bufs=1) as cmtmp:
                        m32 = cmtmp.tile([128, 128], F32, tag="m32")
                        nc.gpsimd.memset(m32[:], 1.0)
                        nc.gpsimd.affine_select(
                            out=m32[:], in_=m32[:],
                            compare_op=mybir.AluOpType.is_ge, fill=0.0,
                            base=0, pattern=[[1, 128]],
                            channel_multiplier=-1)
                        ctri = cpool.tile([128, 128], BF16, tag="ctri")
                        nc.vector.tensor_copy(ctri[:], m32[:])

                def attend_head(b, h, attnT, acc_vec_only=False):
                    t0 = b * c.S
                    q_sb = inpool.tile([128, c.S], BF16, tag="q")
                    k_sb = inpool.tile([128, c.S], BF16, tag="k")
                    v_sb = inpool.tile([128, c.S], BF16, tag="v")
                    nc.sync.dma_start(q_sb[:], qkv[h, :, t0:t0 + c.S])
                    nc.sync.dma_start(k_sb[:], qkv[c.HL + h, :, t0:t0 + c.S])
                    nc.sync.dma_start(v_sb[:],
                                      qkv[2 * c.HL + h, :, t0:t0 + c.S])
                    # V to token-major [tok_p, st, dh]
                    v_tok = vpool.tile([128, c.SKT, 128], BF16, tag="vt")
                    for st in range(c.SKT):
                        vt_ps = ps_ms.tile([128, 128], BF16, tag="ms")
                        nc.tensor.matmul(vt_ps[:],
                                         v_sb[:, st * 128:(st + 1) * 128],
                                         ident[:], is_transpose=True)
                        nc.vector.tensor_copy(v_tok[:, st, :], vt_ps[:])

                    for qc in range(c.QC):
                        nkt = 4 * (qc + 1) if mode == "causal" else c.SKT
                        acc_g = accpool.tile([128, 512], F32R, tag="accg")
                        acc_v = accpool.tile([128, 512], F32R, tag="accv")
                        pts = []
                        for kt in range(nkt):
                            off = kt - 4 * qc  # >=0: diagonal tile (causal)
                            pt = ptpool.tile([128, 512], BF16, tag="pt")
                            s_ps = ps_s.tile([128, 512], F32, tag="s")
                            if mode == "causal" and off > 0:
                                # valid q range is [off*128, 512)
                                w = 512 - off * 128
                                nc.tensor.matmul(
                                    s_ps[:, :w],
                                    k_sb[:, kt * 128:(kt + 1) * 128],
                                    q_sb[:, qc * 512 + off * 128:
                                         (qc + 1) * 512],
                                    start=True, stop=True)
                                nc.vector.memset(pt[:, :off * 128], 0.0)
                                nc.scalar.activation(
                                    pt[:, off * 128:], s_ps[:, :w],
                                    mybir.ActivationFunctionType.Exp,
                                    scale=inv_sqrt_dh)
                                nc.vector.tensor_tensor(
                                    pt[:, off * 128:(off + 1) * 128],
                                    pt[:, off * 128:(off + 1) * 128],
                                    ctri[:], mybir.AluOpType.mult)
                            else:
                                nc.tensor.matmul(
                                    s_ps[:],
                                    k_sb[:, kt * 128:(kt + 1) * 128],
                                    q_sb[:, qc * 512:(qc + 1) * 512],
                                    start=True, stop=True)
                                if mode == "masked":
                                    m_sb = mspool.tile([128, 512], F32,
                                                       tag="m")
                                    nc.sync.dma_start(
                                        m_sb[:],
                                        mask_ext[kt * 128:(kt + 1) * 128,
                                                 qc * 512:(qc + 1) * 512])
                                    nc.vector.tensor_tensor(
                                        s_ps[:], s_ps[:], m_sb[:],
                                        mybir.AluOpType.add)
                                nc.scalar.activation(
                                    pt[:], s_ps[:],
                                    mybir.ActivationFunctionType.Exp,
                                    scale=inv_sqrt_dh)
                                if mode == "causal" and off == 0:
                                    nc.vector.tensor_tensor(
                                        pt[:, :128], pt[:, :128],
                                        ctri[:], mybir.AluOpType.mult)
                            # denominator: two parallel accumulation chains
                            if kt % 2 == 0 and not acc_vec_only:
                                if kt == 0:
                                    nc.gpsimd.tensor_copy(acc_g[:], pt[:])
                                else:
                                    nc.gpsimd.tensor_tensor(
                                        acc_g[:], acc_g[:], pt[:],
                                        mybir.AluOpType.add)
                            else:
                                if kt == (0 if acc_vec_only else 1):
                                    nc.vector.tensor_copy(acc_v[:], pt[:])
                                else:
                                    nc.vector.tensor_tensor(
                                        acc_v[:], acc_v[:], pt[:],
                                        mybir.AluOpType.add)
                            pts.append(pt)
                        # PV first: keeps the in-order tensor queue busy
                        at_ps = ps_at.tile([128, 512], F32, tag="at")
                        for kt in range(nkt):
                            nc.tensor.matmul(
                                at_ps[:], v_tok[:, kt, :], pts[kt][:],
                                start=(kt == 0), stop=(kt == nkt - 1))
                        # denom row -> fast reciprocal -> broadcast
                        if not acc_vec_only:
                            nc.vector.tensor_tensor(
                                acc_v[:], acc_v[:], acc_g[:],
                                mybir.AluOpType.add)
                        den_ps = ps_ms.tile([1, 512], F32, tag="ms")
                        nc.tensor.matmul(den_ps[:], ones_col[:], acc_v[:],
                                         start=True, stop=True)
                        rden = smpool.tile([1, 512], F32, tag="rden")
                        nc.vector.reciprocal_approx_fast(rden[:], den_ps[:])
                        den_r = smpool.tile([1, 512], F32R, tag="denr")
                        nc.vector.tensor_copy(den_r[:], rden[:])
                        bc_ps = ps_ms.tile([128, 512], F32, tag="ms")
                        nc.tensor.matmul(bc_ps[:], ones_row[:], den_r[:],
                                         start=True, stop=True)
                        bc_sb = smpool.tile([128, 512], F32, tag="bc")
                        nc.vector.tensor_copy(bc_sb[:], bc_ps[:])
                        nc.vector.tensor_tensor(
                            attnT[:, h, qc * 512:(qc + 1) * 512],
                            at_ps[:], bc_sb[:], mybir.AluOpType.mult)

                def a2a_batch(b, attnT):
                    # AllToAll: features (this core's heads, all tokens) ->
                    # token shard (all features)
                    a2a_in = dram.tile([c.n_cores, c.FO, c.TSH], BF16,
                                       tag=f"a2a_in{b}", name=f"a2a_in{b}")
                    a2a_out = dram.tile([c.n_cores, c.FO, c.TSH], BF16,
                                        tag=f"a2a_out{b}", name=f"a2a_out{b}")
                    for g in range(c.n_cores):
                        nc.sync.dma_start(
                            a2a_in[g].rearrange("(ft p) t -> p ft t", p=128),
                            attnT[:, :, g * c.TSH:(g + 1) * c.TSH])
                    nc.gpsimd.collective_compute(
                        "AllToAll",
                        mybir.AluOpType.bypass,
                        replica_groups=[list(range(c.n_cores))],
                        ins=[a2a_in[:].opt()],
                        outs=[a2a_out[:].opt()],
                    )
                    # gathered activations: [5120 feats, TSH tokens].
                    # Issued from gpsimd: the wait on the collective stays on
                    # the queue that runs it, not on the DMA-prefetch queue.
                    attn_sb = apool.tile([128, c.KC, c.TSH], BF16, tag="ag")
                    nc.gpsimd.dma_start(
                        attn_sb[:],
                        a2a_out.rearrange("s (ft p) t -> p (s ft) t", p=128))
                    return attn_sb

                KH = c.KC // 2

                def o_proj_chunk(b, attn_sb, oc):
                    wo_sbs = []
                    for half in range(2):
                        wo_sb = wopool.tile([128, KH, 512], BF16, tag="wo")
                        nc.sync.dma_start(
                            wo_sb[:],
                            wo_r[:, half * KH:(half + 1) * KH,
                                 oc * 512:(oc + 1) * 512])
                        wo_sbs.append(wo_sb)
                    for tt in range(c.TSH // 128):
                        ps = ps_op.tile([128, 512], F32, tag="ops")
                        for half in range(2):
                            for k in range(KH):
                                nc.tensor.matmul(
                                    ps[:],
                                    attn_sb[:, half * KH + k,
                                            tt * 128:(tt + 1) * 128],
                                    wo_sbs[half][:, k, :],
                                    start=(half == 0 and k == 0),
                                    stop=(half == 1 and k == KH - 1))
                        po_sb = oopool.tile([128, 512], F32, tag="po")
                        nc.vector.tensor_copy(po_sb[:], ps[:])
                        nc.gpsimd.dma_start(
                            out_ext[b, tt * 128:(tt + 1) * 128,
                                    oc * 512:(oc + 1) * 512],
                            po_sb[:])

                # batch 0: attention, then kick its AllToAll
                attnT0 = atpool.tile([128, c.HL, c.S], BF16, tag="attnT",
                                     name="attnT0")
                for h in range(c.HL):
                    attend_head(0, h, attnT0)
                if dbg:
                    dbg_sb = smpool.tile([128, c.S], F32, tag="dbg")
                    for hh in range(c.HL):
                        nc.vector.tensor_copy(dbg_sb[:], attnT0[:, hh, :])
                        nc.sync.dma_start(dbg_ext[:, hh, :], dbg_sb[:])
                attnT1 = atpool.tile([128, c.HL, c.S], BF16, tag="attnT",
                                     name="attnT1")
                # batch 1 attention interleaved with batch 0 o_proj chunks
                attend_head(1, 0, attnT1, acc_vec_only=True)
                attn_sb0 = a2a_batch(0, attnT0)
                done = 0
                for h in range(1, c.HL):
                    attend_head(1, h, attnT1, acc_vec_only=(h == 1))
                    tgt = (c.OC * h) // (c.HL - 1)
                    while done < tgt:
                        o_proj_chunk(0, attn_sb0, done)
                        done += 1
                while done < c.OC:
                    o_proj_chunk(0, attn_sb0, done)
                    done += 1
                attn_sb1 = a2a_batch(1, attnT1)
                for oc in range(c.OC):
                    o_proj_chunk(1, attn_sb1, oc)

    nc.compile()
    return nc


# --------------------------------------------------------------------------
_CACHE = {}


def _get_program(cfg: Cfg, mode: str):
    key = (cfg.key(), mode)
    if key not in _CACHE:
        _CACHE[key] = build_program(cfg, mode)
    return _CACHE[key]


def prepare_inputs(cfg: Cfg, hidden_states, attention_mask, W_pack, W_o):
    """Host-side shard + layout prep (bf16 cast). Returns (mode, in_maps)."""
    c = cfg
    X = np.asarray(hidden_states, dtype=np.float32).reshape(c.T, c.hidden)
    XT = np.ascontiguousarray(X.T).astype(BF)

    mask = np.asarray(attention_mask, dtype=np.float32).reshape(c.S, c.S)
    causal_ref = np.where(
        np.tril(np.ones((c.S, c.S), dtype=bool)), 0.0, -1e9
    ).astype(np.float32)
    if np.array_equal(mask, causal_ref):
        mode = "causal"
    elif not mask.any():
        mode = "dense"
    else:
        mode = "masked"

    W_pack = np.asarray(W_pack, dtype=np.float32)
    W_o = np.asarray(W_o, dtype=np.float32)
    H = c.hidden
    woT = np.ascontiguousarray(W_o.T).astype(BF)   # [feat, out] full
    in_maps = []
    for g in range(c.n_cores):
        r0, r1 = g * c.FO, (g + 1) * c.FO
        wq = W_pack[r0:r1]
        wk = W_pack[H + r0:H + r1]
        wv = W_pack[2 * H + r0:2 * H + r1]
        wqkvT = np.ascontiguousarray(
            np.concatenate([wq, wk, wv], axis=0).T).astype(BF)  # [H, F]
        m = {"xt": XT, "wqkvt": wqkvT, "wot": woT}
        if mode == "masked":
            m["maskt"] = np.ascontiguousarray(mask.T * math.sqrt(c.dh))
        in_maps.append(m)
    return mode, in_maps


def assemble_output(cfg: Cfg, results):
    c = cfg
    full = np.empty((c.B, c.S, c.hidden), dtype=np.float32)
    for g in range(c.n_cores):
        o = results[g]["out"].reshape(c.B, c.TSH, c.hidden)
        for b in range(c.B):
            full[b, g * c.TSH:(g + 1) * c.TSH] = o[b]
    return full


def kernel(hidden_states, attention_mask, W_pack, W_o):
    cfg = Cfg()
    mode, in_maps = prepare_inputs(cfg, hidden_states, attention_mask,
                                   W_pack, W_o)
    nc = _get_program(cfg, mode)
    res = bass_utils.run_bass_kernel_spmd(nc, in_maps,
                                          list(range(cfg.n_cores)))
    return assemble_output(cfg, res.results)


# revision 24
# speedup vs baseline: 1.5221x; 1.0145x over previous
"""Trainium2 Bass kernel for BaichuanAttention (hidden=5120, 40 heads, b=2, s=2048).

Tensor-parallel over heads across 8 NeuronCores, bf16 compute:
  A) QKV projection with SBUF-resident bf16 weights, X streamed.
  B) Flash-style causal attention in S^T form (scores computed as K^T.Q so
     exp() writes P^T directly -- no P transposes), V transposed on-chip.
  C) Per-batch AllToAll of the small pre-o_proj activations (features ->
     token shards), then a local full-width o_proj per core.
Host reassembles the token-sharded outputs.
"""

import math
import sys

for _p in ("/opt/trn_rl_repo",):
    if _p not in sys.path:
        sys.path.insert(0, _p)

import numpy as np
import ml_dtypes

import concourse.bass as bass
import concourse.mybir as mybir
import concourse.tile as tile
from concourse import bacc, bass_utils

F32 = mybir.dt.float32
F32R = mybir.dt.float32r
BF16 = mybir.dt.bfloat16
BF = ml_dtypes.bfloat16


class Cfg:
    def __init__(self, hidden=5120, n_heads=40, dh=128, B=2, S=2048, n_cores=8):
        self.hidden = hidden
        self.n_heads = n_heads
        self.dh = dh
        self.B = B
        self.S = S
        self.n_cores = n_cores
        assert dh == 128
        self.HL = n_heads // n_cores          # heads per core (5)
        self.F = 3 * self.HL * dh             # per-core packed qkv rows (1920)
        self.FO = self.HL * dh                # per-core attn feature width (640)
        self.T = B * S                        # total tokens (4096)
        self.KC = hidden // 128               # contraction chunks (40)
        self.TC = self.T // 512               # token chunks for qkv (8)
        self.SKT = S // 128                   # k tiles per batch seq (16)
        self.QC = S // 512                    # q chunks per batch (4)
        self.NFT = self.F // 128              # qkv feature tiles (15)
        self.TSH = S // n_cores               # token shard per core per batch (256)
        self.OC = hidden // 512               # o_proj out chunks (10)

    def key(self):
        return (self.hidden, self.n_heads, self.dh, self.B, self.S, self.n_cores)


def build_program(cfg: Cfg, mode: str, dbg: bool = False):
    """mode: 'causal' (mult-mask diag blocks + block skip), 'dense' (no mask),
    'masked' (general additive mask, host passes maskT pre-scaled)."""
    assert mode in ("causal", "dense", "masked")
    c = cfg
    nc = bacc.Bacc("TRN2", target_bir_lowering=False, debug=False,
                   num_devices=c.n_cores)
    dbg_ext = None
    if dbg:
        dbg_ext = nc.dram_tensor("dbg", [128, c.HL, c.S], F32,
                                 kind="ExternalOutput").ap()

    xt = nc.dram_tensor("xt", [c.hidden, c.T], BF16, kind="ExternalInput").ap()
    wqkvt = nc.dram_tensor("wqkvt", [c.hidden, c.F], BF16,
                           kind="ExternalInput").ap()
    wot = nc.dram_tensor("wot", [c.hidden, c.hidden], BF16,
                         kind="ExternalInput").ap()
    mask_ext = None
    if mode == "masked":
        mask_ext = nc.dram_tensor("maskt", [c.S, c.S], F32,
                                  kind="ExternalInput").ap()
    # per-core output: for each batch, this core's token shard (all hidden)
    out_ext = nc.dram_tensor("out", [c.B, c.TSH, c.hidden], F32,
                             kind="ExternalOutput").ap()

    inv_sqrt_dh = 1.0 / math.sqrt(c.dh)

    xt_r = xt.rearrange("(kc p) t -> p kc t", p=128)
    wq_r = wqkvt.rearrange("(kc p) f -> p kc f", p=128)
    wo_r = wot.rearrange("(kc p) j -> p kc j", p=128)

    with tile.TileContext(nc) as tc:
        with tc.tile_pool(name="dram", bufs=1, space="DRAM") as dram:
            qkv = dram.tile([c.NFT, 128, c.T], BF16)

            # ---------------- Phase A: QKV projection -------------------
            # qkv[ft, d, t] = sum_h W[h, ft*128+d] * X[h, t]   (feature-major)
            splits = [8, 7] if c.NFT == 15 else [c.NFT]
            with tc.tile_pool(name="qkv_w", bufs=1) as wpool, \
                 tc.tile_pool(name="qkv_x", bufs=2) as xpool, \
                 tc.tile_pool(name="qkv_o", bufs=8) as opool, \
                 tc.tile_pool(name="qkv_ps", bufs=8, space="PSUM") as pspool:
                assert c.KC % 4 == 0
                KQ = c.KC // 4
                ft0 = 0
                for nft in splits:
                    wts = None
                    for tci in range(c.TC):
                        xq = [xpool.tile([128, KQ, 512], BF16, tag=f"x{j}",
                                         name=f"x{j}") for j in range(4)]
                        for j in range(4):
                            nc.sync.dma_start(
                                xq[j][:],
                                xt_r[:, j * KQ:(j + 1) * KQ,
                                     tci * 512:(tci + 1) * 512])
                        if tci == 0:
                            # per-kc weight tiles: lets the next group's
                            # weight loads overlap this group's tail
                            wts = []
                            for kc in range(c.KC):
                                w_t = wpool.tile([128, max(splits) * 128],
                                                 BF16, tag=f"w{kc}",
                                                 name=f"w{kc}")
                                nc.sync.dma_start(
                                    w_t[:, :nft * 128],
                                    wq_r[:, kc,
                                         ft0 * 128:(ft0 + nft) * 128])
                                wts.append(w_t)
                        pss = [pspool.tile([128, 512], F32, tag="ps",
                                           name=f"ps{i}")
                               for i in range(nft)]
                        for kc in range(c.KC):
                            for i in range(nft):
                                nc.tensor.matmul(
                                    pss[i][:],
                                    wts[kc][:, i * 128:(i + 1) * 128],
                                    xq[kc // KQ][:, kc % KQ, :],
                                    start=(kc == 0), stop=(kc == c.KC - 1))
                        for i in range(nft):
                            o_sb = opool.tile([128, 512], BF16, tag="o")
                            nc.vector.tensor_copy(o_sb[:], pss[i][:])
                            nc.sync.dma_start(
                                qkv[ft0 + i, :, tci * 512:(tci + 1) * 512],
                                o_sb[:])
                    ft0 += nft

            # ---------------- Phase B + C (interleaved per batch) -------
            with tc.tile_pool(name="att_const", bufs=1) as cpool, \
                 tc.tile_pool(name="att_at", bufs=2) as atpool, \
                 tc.tile_pool(name="att_in", bufs=3) as inpool, \
                 tc.tile_pool(name="att_v", bufs=2) as vpool, \
                 tc.tile_pool(name="att_pt", bufs=c.SKT + 1) as ptpool, \
                 tc.tile_pool(name="att_acc", bufs=2) as accpool, \
                 tc.tile_pool(name="att_sm", bufs=3) as smpool, \
                 tc.tile_pool(name="att_ms", bufs=(4 if mode == "masked" else 1)) as mspool, \
                 tc.tile_pool(name="op_attn", bufs=1) as apool, \
                 tc.tile_pool(name="op_w", bufs=2) as wopool, \
                 tc.tile_pool(name="op_o", bufs=2) as oopool, \
                 tc.tile_pool(name="ps_s", bufs=2, space="PSUM") as ps_s, \
                 tc.tile_pool(name="ps_at", bufs=2, space="PSUM") as ps_at, \
                 tc.tile_pool(name="ps_ms", bufs=2, space="PSUM") as ps_ms, \
                 tc.tile_pool(name="ps_op", bufs=2, space="PSUM") as ps_op:

                # constants
                ident = cpool.tile([128, 128], BF16)
                ones_col = cpool.tile([128, 1], F32R)   # lhsT for colsum
                ones_row = cpool.tile([1, 128], F32R)   # lhsT for broadcast
                with tc.tile_pool(name="att_tmp", bufs=1) as tmppool:
                    t32 = tmppool.tile([128, 128], F32)
                    nc.gpsimd.memset(t32[:], 0.0)
                    nc.gpsimd.affine_select(
                        out=t32[:], in_=t32[:],
                        compare_op=mybir.AluOpType.not_equal, fill=1.0,
                        base=0, pattern=[[-1, 128]], channel_multiplier=1)
                    nc.vector.tensor_copy(ident[:], t32[:])
                    o32 = tmppool.tile([128, 1], F32, tag="o32")
                    nc.vector.memset(o32[:], 1.0)
                    nc.vector.tensor_copy(ones_col[:], o32[:])
                    r32 = tmppool.tile([1, 128], F32, tag="r32")
                    nc.vector.memset(r32[:], 1.0)
                    nc.vector.tensor_copy(ones_row[:], r32[:])
                ctri = None
                if mode == "causal":
                    # multiplicative triangle mask [128k, 128q]:
                    # m[p, y] = 1 where y >= p else 0
                    with tc.tile_pool(name="att_cm", bufs=1) as cmtmp:
                        m32 = cmtmp.tile([128, 128], F32, tag="m32")
                        nc.gpsimd.memset(m32[:], 1.0)
                        nc.gpsimd.affine_select(
                            out=m32[:], in_=m32[:],
                            compare_op=mybir.AluOpType.is_ge, fill=0.0,
                            base=0, pattern=[[1, 128]],
                            channel_multiplier=-1)
                        ctri = cpool.tile([128, 128], BF16, tag="ctri")
                        nc.vector.tensor_copy(ctri[:], m32[:])

                pending = []

                def flush_tail():
                    # softmax tail of the previous q-chunk, emitted late so
                    # its cross-engine waits hide under the next chunk's work
                    if not pending:
                        return
                    (attnT_p, h_p, qc_p, acc_v, acc_g, at_ps, vec_only) = \
                        pending.pop()
                    if not vec_only:
                        nc.vector.tensor_tensor(acc_v[:], acc_v[:], acc_g[:],
                                                mybir.AluOpType.add)
                    den_ps = ps_ms.tile([1, 512], F32, tag="ms")
                    nc.tensor.matmul(den_ps[:], ones_col[:], acc_v[:],
                                     start=True, stop=True)
                    rden = smpool.tile([1, 512], F32, tag="rden")
                    nc.vector.reciprocal_approx_fast(rden[:], den_ps[:])
                    den_r = smpool.tile([1, 512], F32R, tag="denr")
                    nc.vector.tensor_copy(den_r[:], rden[:])
                    bc_ps = ps_ms.tile([128, 512], F32, tag="ms")
                    nc.tensor.matmul(bc_ps[:], ones_row[:], den_r[:],
                                     start=True, stop=True)
                    bc_sb = smpool.tile([128, 512], F32, tag="bc")
                    nc.vector.tensor_copy(bc_sb[:], bc_ps[:])
                    nc.vector.tensor_tensor(
                        attnT_p[:, h_p, qc_p * 512:(qc_p + 1) * 512],
                        at_ps[:], bc_sb[:], mybir.AluOpType.mult)

                def attend_head(b, h, attnT, acc_vec_only=False):
                    t0 = b * c.S
                    q_sb = inpool.tile([128, c.S], BF16, tag="q")
                    k_sb = inpool.tile([128, c.S], BF16, tag="k")
                    v_sb = inpool.tile([128, c.S], BF16, tag="v")
                    nc.sync.dma_start(q_sb[:], qkv[h, :, t0:t0 + c.S])
                    nc.sync.dma_start(k_sb[:], qkv[c.HL + h, :, t0:t0 + c.S])
                    nc.sync.dma_start(v_sb[:],
                                      qkv[2 * c.HL + h, :, t0:t0 + c.S])
                    # V to token-major [tok_p, st, dh]
                    v_tok = vpool.tile([128, c.SKT, 128], BF16, tag="vt")
                    for st in range(c.SKT):
                        vt_ps = ps_ms.tile([128, 128], BF16, tag="ms")
                        nc.tensor.matmul(vt_ps[:],
                                         v_sb[:, st * 128:(st + 1) * 128],
                                         ident[:], is_transpose=True)
                        nc.vector.tensor_copy(v_tok[:, st, :], vt_ps[:])

                    for qc in range(c.QC):
                        nkt = 4 * (qc + 1) if mode == "causal" else c.SKT
                        acc_g = accpool.tile([128, 512], F32R, tag="accg")
                        acc_v = accpool.tile([128, 512], F32R, tag="accv")
                        pts = []
                        for kt in range(nkt):
                            off = kt - 4 * qc  # >=0: diagonal tile (causal)
                            pt = ptpool.tile([128, 512], BF16, tag="pt")
                            s_ps = ps_s.tile([128, 512], F32, tag="s")
                            if mode == "causal" and off > 0:
                                # valid q range is [off*128, 512)
                                w = 512 - off * 128
                                nc.tensor.matmul(
                                    s_ps[:, :w],
                                    k_sb[:, kt * 128:(kt + 1) * 128],
                                    q_sb[:, qc * 512 + off * 128:
                                         (qc + 1) * 512],
                                    start=True, stop=True)
                                nc.vector.memset(pt[:, :off * 128], 0.0)
                                nc.scalar.activation(
                                    pt[:, off * 128:], s_ps[:, :w],
                                    mybir.ActivationFunctionType.Exp,
                                    scale=inv_sqrt_dh)
                                nc.vector.tensor_tensor(
                                    pt[:, off * 128:(off + 1) * 128],
                                    pt[:, off * 128:(off + 1) * 128],
                                    ctri[:], mybir.AluOpType.mult)
                            else:
                                nc.tensor.matmul(
                                    s_ps[:],
                                    k_sb[:, kt * 128:(kt + 1) * 128],
                                    q_sb[:, qc * 512:(qc + 1) * 512],
                                    start=True, stop=True)
                                if mode == "masked":
                                    m_sb = mspool.tile([128, 512], F32,
                                                       tag="m")
                                    nc.sync.dma_start(
                                        m_sb[:],
                                        mask_ext[kt * 128:(kt + 1) * 128,
                                                 qc * 512:(qc + 1) * 512])
                                    nc.vector.tensor_tensor(
                                        s_ps[:], s_ps[:], m_sb[:],
                                        mybir.AluOpType.add)
                                nc.scalar.activation(
                                    pt[:], s_ps[:],
                                    mybir.ActivationFunctionType.Exp,
                                    scale=inv_sqrt_dh)
                                if mode == "causal" and off == 0:
                                    nc.vector.tensor_tensor(
                                        pt[:, :128], pt[:, :128],
                                        ctri[:], mybir.AluOpType.mult)
                            # denominator: two parallel accumulation chains
                            if kt % 2 == 0 and not acc_vec_only:
                                if kt == 0:
                                    nc.gpsimd.tensor_copy(acc_g[:], pt[:])
                                else:
                                    nc.gpsimd.tensor_tensor(
                                        acc_g[:], acc_g[:], pt[:],
                                        mybir.AluOpType.add)
                            else:
                                if kt == (0 if acc_vec_only else 1):
                                    nc.vector.tensor_copy(acc_v[:], pt[:])
                                else:
                                    nc.vector.tensor_tensor(
                                        acc_v[:], acc_v[:], pt[:],
                                        mybir.AluOpType.add)
                            pts.append(pt)
                        # PV
                        at_ps = ps_at.tile([128, 512], F32, tag="at")
                        for kt in range(nkt):
                            nc.tensor.matmul(
                                at_ps[:], v_tok[:, kt, :], pts[kt][:],
                                start=(kt == 0), stop=(kt == nkt - 1))
                        # softmax tail of the PREVIOUS chunk, now that its
                        # inputs are long ready; ours is deferred
                        flush_tail()
                        pending.append((attnT, h, qc, acc_v, acc_g, at_ps,
                                        acc_vec_only))

                def a2a_batch(b, attnT):
                    # AllToAll: features (this core's heads, all tokens) ->
                    # token shard (all features). Split into 128-token parts
                    # so downstream o_proj can start on part 0 earlier.
                    flush_tail()
                    parts = []
                    for p in range(c.TSH // 128):
                        a2a_in = dram.tile([c.n_cores, c.FO, 128], BF16,
                                           tag=f"a2a_in{b}{p}",
                                           name=f"a2a_in{b}{p}")
                        a2a_out = dram.tile([c.n_cores, c.FO, 128], BF16,
                                            tag=f"a2a_out{b}{p}",
                                            name=f"a2a_out{b}{p}")
                        for g in range(c.n_cores):
                            o0 = g * c.TSH + p * 128
                            nc.sync.dma_start(
                                a2a_in[g].rearrange("(f q) t -> q f t",
                                                    q=128),
                                attnT[:, :, o0:o0 + 128])
                        nc.gpsimd.collective_compute(
                            "AllToAll",
                            mybir.AluOpType.bypass,
                            replica_groups=[list(range(c.n_cores))],
                            ins=[a2a_in[:].opt()],
                            outs=[a2a_out[:].opt()],
                        )
                        # gathered activations [5120 feats, 128 tokens];
                        # issued from gpsimd so the collective wait stays off
                        # the DMA-prefetch queue
                        attn_sb = apool.tile([128, c.KC, 128], BF16,
                                             tag=f"ag{p}", name=f"ag{b}{p}")
                        nc.gpsimd.dma_start(
                            attn_sb[:],
                            a2a_out.rearrange("s (f q) t -> q (s f) t",
                                              q=128))
                        parts.append(attn_sb)
                    return parts

                KH = c.KC // 2

                def o_proj_chunk(b, parts, oc):
                    wo_sbs = []
                    for half in range(2):
                        wo_sb = wopool.tile([128, KH, 512], BF16, tag="wo")
                        nc.sync.dma_start(
                            wo_sb[:],
                            wo_r[:, half * KH:(half + 1) * KH,
                                 oc * 512:(oc + 1) * 512])
                        wo_sbs.append(wo_sb)
                    for tt, attn_sb in enumerate(parts):
                        ps = ps_op.tile([128, 512], F32, tag="ops")
                        for half in range(2):
                            for k in range(KH):
                                nc.tensor.matmul(
                                    ps[:],
                                    attn_sb[:, half * KH + k, :],
                                    wo_sbs[half][:, k, :],
                                    start=(half == 0 and k == 0),
                                    stop=(half == 1 and k == KH - 1))
                        po_sb = oopool.tile([128, 512], F32, tag="po")
                        nc.vector.tensor_copy(po_sb[:], ps[:])
                        nc.gpsimd.dma_start(
                            out_ext[b, tt * 128:(tt + 1) * 128,
                                    oc * 512:(oc + 1) * 512],
                            po_sb[:])

                # batch 0: attention, then kick its AllToAll
                attnT0 = atpool.tile([128, c.HL, c.S], BF16, tag="attnT",
                                     name="attnT0")
                for h in range(c.HL):
                    attend_head(0, h, attnT0)
                if dbg:
                    flush_tail()
                    dbg_sb = smpool.tile([128, c.S], F32, tag="dbg")
                    for hh in range(c.HL):
                        nc.vector.tensor_copy(dbg_sb[:], attnT0[:, hh, :])
                        nc.sync.dma_start(dbg_ext[:, hh, :], dbg_sb[:])
                attnT1 = atpool.tile([128, c.HL, c.S], BF16, tag="attnT",
                                     name="attnT1")
                # batch 1 attention interleaved with batch 0 o_proj chunks
                attend_head(1, 0, attnT1, acc_vec_only=True)
                attn_sb0 = a2a_batch(0, attnT0)
                done = 0
                for h in range(1, c.HL):
                    attend_head(1, h, attnT1, acc_vec_only=(h == 1))
                    tgt = (c.OC * h) // (c.HL - 1)
                    while done < tgt:
                        o_proj_chunk(0, attn_sb0, done)
                        done += 1
                while done < c.OC:
                    o_proj_chunk(0, attn_sb0, done)
                    done += 1
                attn_sb1 = a2a_batch(1, attnT1)
                for oc in range(c.OC):
                    o_proj_chunk(1, attn_sb1, oc)

    nc.compile()
    return nc


# --------------------------------------------------------------------------
_CACHE = {}


def _get_program(cfg: Cfg, mode: str):
    key = (cfg.key(), mode)
    if key not in _CACHE:
        _CACHE[key] = build_program(cfg, mode)
    return _CACHE[key]


def prepare_inputs(cfg: Cfg, hidden_states, attention_mask, W_pack, W_o):
    """Host-side shard + layout prep (bf16 cast). Returns (mode, in_maps)."""
    c = cfg
    X = np.asarray(hidden_states, dtype=np.float32).reshape(c.T, c.hidden)
    XT = np.ascontiguousarray(X.T).astype(BF)

    mask = np.asarray(attention_mask, dtype=np.float32).reshape(c.S, c.S)
    causal_ref = np.where(
        np.tril(np.ones((c.S, c.S), dtype=bool)), 0.0, -1e9
    ).astype(np.float32)
    if np.array_equal(mask, causal_ref):
        mode = "causal"
    elif not mask.any():
        mode = "dense"
    else:
        mode = "masked"

    W_pack = np.asarray(W_pack, dtype=np.float32)
    W_o = np.asarray(W_o, dtype=np.float32)
    H = c.hidden
    woT = np.ascontiguousarray(W_o.T).astype(BF)   # [feat, out] full
    in_maps = []
    for g in range(c.n_cores):
        r0, r1 = g * c.FO, (g + 1) * c.FO
        wq = W_pack[r0:r1]
        wk = W_pack[H + r0:H + r1]
        wv = W_pack[2 * H + r0:2 * H + r1]
        wqkvT = np.ascontiguousarray(
            np.concatenate([wq, wk, wv], axis=0).T).astype(BF)  # [H, F]
        m = {"xt": XT, "wqkvt": wqkvT, "wot": woT}
        if mode == "masked":
            m["maskt"] = np.ascontiguousarray(mask.T * math.sqrt(c.dh))
        in_maps.append(m)
    return mode, in_maps


def assemble_output(cfg: Cfg, results):
    c = cfg
    full = np.empty((c.B, c.S, c.hidden), dtype=np.float32)
    for g in range(c.n_cores):
        o = results[g]["out"].reshape(c.B, c.TSH, c.hidden)
        for b in range(c.B):
            full[b, g * c.TSH:(g + 1) * c.TSH] = o[b]
    return full


def kernel(hidden_states, attention_mask, W_pack, W_o):
    cfg = Cfg()
    mode, in_maps = prepare_inputs(cfg, hidden_states, attention_mask,
                                   W_pack, W_o)
    nc = _get_program(cfg, mode)
    res = bass_utils.run_bass_kernel_spmd(nc, in_maps,
                                          list(range(cfg.n_cores)))
    return assemble_output(cfg, res.results)
